# revision 2
# baseline (speedup 1.0000x reference)
"""Trainium2 kernel for nn_MultiHeadClassifier.

Math: out[i] = W[task_labels[i]] @ x[i] + b[task_labels[i]]
  x [262144, 1024] f32, task_labels [262144] int, W [8, 32, 1024], b [8, 32]

Strategy (8 NeuronCores, task-parallel, fp8 x, fp8 out):
  - Host sorts rows by task; core c processes (up to NCAP=32768) rows of
    task c, so W[c] is a per-core constant and there is NO routing on
    device at all — each core runs a plain GEMM. The ~few hundred rows
    that overflow a core's capacity are computed on host (numpy) and the
    result is merged back; bias is added on host.
  - x is sent as fp8 e3m4 (1 byte: 4 mantissa bits), quartering the
    dominant HBM traffic vs f32. W is scaled by 128 and sent as an
    e3m4 hi+lo pair, which cancels the W quantization error to second
    order. The per-core stream is SDMA-engine-bound (~26 GB/s x 16
    engines ~= 416 GB/s), so every byte counts: the output is merged
    on-device (ACT copies hi to an f32 staging tile, DVE adds lo, ACT
    casts x(1/128) to e3m4) and shipped as fp8 — 1.1 MiB instead of the
    2.5 MiB a bf16 + raw-hi/lo scheme costs. Measured rel err 1.76e-2
    vs the 2e-2 gate (HW-validated numpy sim of the exact op chain).
  - hi and lo live side by side in one M=64 stationary [128, 64], so
    each k-tile needs ONE matmul. Two 512-row chunks pack into each
    PSUM bank via column tiling (positions 0/64), which the PE streams
    2-way concurrently (the XBUS ceiling measured on HW) -> ~3.4us of
    PE per 2048-row quad vs ~5.2us of DMA.
  - x quads (2 MB, 16 KB/partition contiguous) stream on the SP HWDGE
    ring; wt is pre-transposed on host so its const DMA is a handful of
    contiguous descriptors. Merged fp8 output ships in 256 KiB groups
    on the ACT ring (2 KiB/partition lines). The last 1024 rows ship
    their PSUM banks raw (bf16, host adds hi+lo) on the idle SP ring so
    the post-stream tail is one copy + one 64 KiB DMA per piece.
"""

import sys

sys.path.insert(0, "/opt/trn_rl_repo")

import numpy as np
import ml_dtypes

import concourse.bass as bass
import concourse.tile as tile
from concourse import bacc, mybir
from concourse import bass_utils

B, D, C, T = 262144, 1024, 32, 8
NCORES = 8
P = 128
KO = D // P  # 8 contraction k-tiles
CH = 512  # rows per chunk (one PSUM column-tile)
QR = 2048  # rows per quad (one x DMA; 4 chunks in 2 PSUM banks)
NQ = 16  # quads per core
NCAP = QR * NQ  # 32768 rows per core capacity
GQ = 4  # quads per output DMA group
WSCALE = 128.0  # power of two; exactly cancelled by the on-device 1/128
INV_WSCALE = 1.0 / WSCALE

F8 = ml_dtypes.float8_e3m4
F8LIM = 15.5  # max finite e3m4

# set by test harness to collect a profile; harness-invoked kernel() keeps it off
TRACE = False
LAST_RESULTS = None
LAST_IN_MAPS = None


def _build():
    f32 = mybir.dt.float32
    bf16 = mybir.dt.bfloat16
    f8 = mybir.dt.float8e3

    nc = bacc.Bacc("TRN2", debug=False, num_devices=NCORES)
    # xt[m, ki, ko, r]: one quad is a contiguous 2 MB region with
    # 16 KB contiguous per partition. Quads 0..14 (15 body quads).
    xt_d = nc.dram_tensor("xt", [NQ - 1, P, KO, QR], f8, kind="ExternalInput")
    # final quad split: first 1024 rows as one 1 MB transfer (merged out),
    xty_d = nc.dram_tensor("xty", [P, KO, 2 * CH], f8, kind="ExternalInput")
    # last 1024 rows as two 0.5 MB transfers, each a 2x256-row pair (raw out)
    xtz_d = nc.dram_tensor("xtz", [2, P, KO, CH], f8, kind="ExternalInput")
    # wt[ki, ko, 0:32]=hi, [ki, ko, 32:64]=lo (host-transposed, scaled)
    wt_d = nc.dram_tensor("wt", [P, KO, 2 * C], f8, kind="ExternalInput")
    # merged fp8 output (already divided by 128 on device):
    #   out_d[g, 32j+c, 512q+r] = quads 0..11 (g=0..2, q=0..3)
    out_d = nc.dram_tensor("out", [3, P, GQ * CH], f8, kind="ExternalOutput")
    #   outm_d[32j+c, 512q+r] = quads 12..14 (q=0..2)
    outm_d = nc.dram_tensor("outm", [P, 3 * CH], f8, kind="ExternalOutput")
    #   outy_d[32jj+c, r] = xty rows (jj=0,1)
    outy_d = nc.dram_tensor("outy", [2 * C, CH], f8, kind="ExternalOutput")
    # last two banks raw (host adds hi+lo): piece k, partition 64h+{c|32+c}
    outz_d = nc.dram_tensor("outz", [2, P, CH // 2], bf16, kind="ExternalOutput")

    with tile.TileContext(nc) as tc:
        with (
            tc.tile_pool(name="consts", bufs=1) as consts,
            tc.tile_pool(name="xpool", bufs=8) as xpool,
            tc.tile_pool(name="tailx", bufs=1) as tailx,
            tc.tile_pool(name="stage", bufs=4) as stage,
            tc.tile_pool(name="opool", bufs=4) as opool,
            tc.tile_pool(name="psum", bufs=6, space="PSUM") as psum,
            tc.tile_pool(name="psumz", bufs=2, space="PSUM") as psumz,
        ):
            # first x quad in flight before the consts
            xq0 = xpool.tile([P, KO, QR], f8, tag="xq")
            nc.sync.dma_start(xq0[:], xt_d[0])

            # consts on the ACT ring (contiguous layout: cheap descriptors)
            wt = consts.tile([P, KO, 2 * C], f8)
            nc.scalar.dma_start(wt[:], wt_d[:])

            # Engine warmups: give PE and DVE one instruction that observes
            # the const DMA lane so steady-state instructions carry at most
            # one sync wait each.
            scratch = psum.tile([P, CH], f32, tag="y4")
            nc.tensor.matmul(
                scratch[:2, :2], wt[:, 0, :2], wt[:, 0, :2], start=True, stop=True
            )
            dve_scr = consts.tile([1, 2 * C], bf16)
            nc.vector.tensor_copy(dve_scr[:], wt[:1, 0, :])

            def merge_chunks(banks, dst_f32, n):
                # chunk j of this quad: bank j//2, partitions 64*(j%2)+{hi,lo}
                # ACT copies hi (PSUM->f32 SBUF), DVE adds lo
                # (an instruction may read at most one PSUM operand)
                for j in range(n):
                    bank = banks[j // 2]
                    base = 64 * (j % 2)
                    sl = dst_f32[C * j : C * (j + 1), :]
                    nc.scalar.copy(sl, bank[base : base + C, :])
                    nc.vector.tensor_tensor(
                        sl, sl, bank[base + C : base + 2 * C, :], mybir.AluOpType.add
                    )

            for m in range(NQ):
                if m == NQ - 1:
                    # final quad: 1 MB merged half + two raw 0.5 MB pieces.
                    xty = tailx.tile([P, KO, 2 * CH], f8)
                    nc.sync.dma_start(xty[:], xty_d[:])
                    xz0 = tailx.tile([P, KO, CH], f8)
                    nc.sync.dma_start(xz0[:], xtz_d[0])
                    xz1 = tailx.tile([P, KO, CH], f8)
                    nc.sync.dma_start(xz1[:], xtz_d[1])
                    # xty: 2 chunks in one bank (positions 0/64), merged fp8
                    ya = psum.tile([P, CH], f32, tag="y4")
                    for ko in range(KO):
                        for pos, cix in ((0, 0), (64, 1)):
                            nc.tensor.matmul(
                                ya[pos : pos + 2 * C, :],
                                wt[:, ko, :],
                                xty[:, ko, CH * cix : CH * (cix + 1)],
                                start=(ko == 0),
                                stop=(ko == KO - 1),
                                tile_position=(0, pos),
                                skip_group_check=True,
                            )
                    sty = stage.tile([2 * C, CH], f32, tag="sty")
                    for jj in range(2):
                        sl = sty[C * jj : C * (jj + 1), :]
                        nc.scalar.copy(sl, ya[64 * jj : 64 * jj + C, :])
                        nc.vector.tensor_tensor(
                            sl,
                            sl,
                            ya[64 * jj + C : 64 * jj + 2 * C, :],
                            mybir.AluOpType.add,
                        )
                    oy = opool.tile([2 * C, CH], f8, tag="oy")
                    nc.scalar.mul(oy[:], sty[:], INV_WSCALE)
                    nc.sync.dma_start(outy_d[:], oy[:])
                    # last 1024 rows: each 0.5 MB transfer computes as a
                    # 2x256-row pair (N=256 keeps 2-way MM concurrency);
                    # banks ship raw bf16 (host adds hi+lo) on the SP ring
                    for k, (srcx, eng) in enumerate(
                        ((xz0, nc.scalar), (xz1, nc.vector))
                    ):
                        bank = psumz.tile([P, CH // 2], f32, tag="yz")
                        for ko in range(KO):
                            for pos in (0, 64):
                                nc.tensor.matmul(
                                    bank[pos : pos + 2 * C, :],
                                    wt[:, ko, :],
                                    srcx[
                                        :,
                                        ko,
                                        CH // 2 * (pos // 64) : CH // 2 * (pos // 64)
                                        + CH // 2,
                                    ],
                                    start=(ko == 0),
                                    stop=(ko == KO - 1),
                                    tile_position=(0, pos),
                                    skip_group_check=True,
                                )
                        zseg = opool.tile([P, CH // 2], bf16, tag="piece")
                        if eng is nc.scalar:
                            nc.scalar.copy(zseg[:], bank[:])
                        else:
                            nc.vector.tensor_copy(zseg[:], bank[:])
                        nc.sync.dma_start(outz_d[k], zseg[:])
                    continue
                g, q = m // GQ, m % GQ
                if m == 0:
                    xq = xq0
                else:
                    xq = xpool.tile([P, KO, QR], f8, tag="xq")
                    # all x on the SP ring: the ACT sequencer is busy with
                    # hi-copies, and x triggers must never queue behind them
                    nc.sync.dma_start(xq[:], xt_d[m])
                if q == 0:
                    ncols = GQ * CH if g < 3 else 3 * CH
                    out_g = opool.tile([P, ncols], f8, tag="out")
                # 2 banks x 2 column positions = 4 chunks of 512 rows
                ya = psum.tile([P, CH], f32, tag="y4")
                yb = psum.tile([P, CH], f32, tag="y4")
                for ko in range(KO):
                    for bank, (j0, j1) in ((ya, (0, 1)), (yb, (2, 3))):
                        for pos, j in ((0, j0), (64, j1)):
                            nc.tensor.matmul(
                                bank[pos : pos + 2 * C, :],
                                wt[:, ko, :],
                                xq[:, ko, CH * j : CH * (j + 1)],
                                start=(ko == 0),
                                stop=(ko == KO - 1),
                                tile_position=(0, pos),
                                skip_group_check=True,
                            )
                st = stage.tile([P, CH], f32, tag="st")
                merge_chunks((ya, yb), st, 4)
                # scaled cast to fp8 in one 128-partition ACT op
                nc.scalar.mul(out_g[:, CH * q : CH * (q + 1)], st[:], INV_WSCALE)
                if g < 3 and q == GQ - 1:
                    nc.scalar.dma_start(out_d[g], out_g[:])
                elif g == 3 and q == 2:
                    # last merged group (quads 12..14) ships before the
                    # tail pieces, on the ACT ring
                    nc.scalar.dma_start(outm_d[:], out_g[:])
    nc.compile()
    return nc


_NC = None


def _get_nc():
    global _NC
    if _NC is None:
        _NC = _build()
    return _NC


def kernel(x, task_labels, W, b):
    global LAST_RESULTS, LAST_IN_MAPS
    x = np.asarray(x)
    if x.dtype != np.float32:
        x = x.astype(np.float32)
    labels = np.asarray(task_labels).astype(np.int64)
    W = np.asarray(W)
    if W.dtype != np.float32:
        W = W.astype(np.float32)
    b = np.asarray(b)
    if b.dtype != np.float32:
        b = b.astype(np.float32)

    order = np.argsort(labels, kind="stable")  # rows grouped by task
    counts = np.bincount(labels, minlength=T)
    starts = np.concatenate([[0], np.cumsum(counts)])

    in_maps = []
    over_rows = []  # (task, global row indices beyond capacity)
    for t in range(T):
        seg_idx = order[starts[t] : starts[t + 1]]
        n_dev = min(counts[t], NCAP)
        xs = np.zeros((NCAP, D), dtype=F8)
        xs[:n_dev] = x[seg_idx[:n_dev]]
        # xt[m, ki, ko, r] = xs[m*QR + r, ko*P + ki]
        xt = np.ascontiguousarray(
            xs[: (NQ - 1) * QR].reshape(NQ - 1, QR, KO, P).transpose(0, 3, 2, 1)
        )
        xty = np.ascontiguousarray(
            xs[(NQ - 1) * QR : NCAP - 2 * CH]
            .reshape(2 * CH, KO, P)
            .transpose(2, 1, 0)
        )
        xtz = np.ascontiguousarray(
            xs[NCAP - 2 * CH :].reshape(2, CH, KO, P).transpose(0, 3, 2, 1)
        )
        ws = W[t].astype(np.float64) * WSCALE
        hi = np.clip(ws, -F8LIM, F8LIM).astype(F8)
        lo = np.clip(ws - hi.astype(np.float64), -F8LIM, F8LIM).astype(F8)
        # wt[ki, ko, 0:32]=hi[c, ko*128+ki], [ki, ko, 32:64]=lo
        wt = np.empty((P, KO, 2 * C), dtype=F8)
        wt[:, :, :C] = hi.T.reshape(KO, P, C).transpose(1, 0, 2)
        wt[:, :, C:] = lo.T.reshape(KO, P, C).transpose(1, 0, 2)
        in_maps.append(
            {"xt": xt, "xty": xty, "xtz": xtz, "wt": np.ascontiguousarray(wt)}
        )
        if counts[t] > NCAP:
            over_rows.append((t, seg_idx[NCAP:]))

    LAST_IN_MAPS = in_maps
    nc = _get_nc()
    res = bass_utils.run_bass_kernel_spmd(
        nc, in_maps, core_ids=list(range(NCORES)), trace=TRACE
    )
    LAST_RESULTS = res

    out = np.empty((B, C), dtype=np.float32)
    for t in range(T):
        seg_idx = order[starts[t] : starts[t + 1]]
        n_dev = min(counts[t], NCAP)
        o = np.empty((NCAP, C), dtype=np.float32)
        # out_d[g, 32j+c, 512q+r] -> row 2048*(4g+q) + 512*j + r
        og = np.asarray(res.results[t]["out"]).astype(np.float32)
        o[: 12 * QR] = (
            og.reshape(3, 4, C, GQ, CH).transpose(0, 3, 1, 4, 2).reshape(12 * QR, C)
        )
        # outm[32j+c, 512q+r] -> row 2048*(12+q) + 512*j + r
        om = np.asarray(res.results[t]["outm"]).astype(np.float32)
        o[12 * QR : 15 * QR] = (
            om.reshape(4, C, 3, CH).transpose(2, 0, 3, 1).reshape(3 * QR, C)
        )
        # outy[32jj+c, r] -> row 30720 + 512*jj + r
        oy = np.asarray(res.results[t]["outy"]).astype(np.float32)
        o[15 * QR : 15 * QR + 2 * CH] = (
            oy.reshape(2, C, CH).transpose(0, 2, 1).reshape(2 * CH, C)
        )
        # outz[k, 64h+{c|32+c}, r] raw: row 31744 + 512k + 256h + r,
        # value = (hi + lo) / 128
        oz = np.asarray(res.results[t]["outz"]).astype(np.float32)
        ozr = oz.reshape(2, 2, 2, C, CH // 2)  # [k, h, hi/lo, c, r]
        zm = (ozr[:, :, 0] + ozr[:, :, 1]) * np.float32(INV_WSCALE)
        o[NCAP - 2 * CH :] = zm.transpose(0, 1, 3, 2).reshape(2 * CH, C)
        out[seg_idx[:n_dev]] = o[:n_dev]
    for t, idx in over_rows:
        out[idx] = x[idx] @ W[t].T
    out += b[labels]
    return out
